# revision 23
# baseline (speedup 1.0000x reference)
"""Trainium2 Bass kernel for nn_DetectionLoss (focal BCE + online hard negative mining).

Contract: kernel(**inputs) takes FULL inputs (pred/target/mask_ignore/neg_rand,
each [64, 110592, 1] f32) and returns the full output (cls_pos_loss, cls_neg_loss)
as two f32 scalars, matching the jax reference.

Sharding: pure data parallel over the batch dim — 8 samples per NeuronCore,
8 cores. Each core computes per-sample (pos_loss, neg_loss); the host sums the
64 pairs and divides by B (the all-reduce of two scalars).

Device algorithm per sample (layout: sample = 16 partitions x 6912 cols;
8 samples stacked -> [128, 6912] per core):
  stream (3 ACT-table sweeps, fenced so each table loads exactly once):
    A:  p = sigmoid(x)
        wn  = p^2 * (0.25 + 0.125*(clip(5(p-.5),0,1) + 1[x>0])) * 1[g==0]
        wpx = (1-p)^2 * t * (1 + 3*1[x<ln4])
        u_eff = u + 4t;  num_pos = sum(t)
    B1: e = exp(-|x|)                       (staged full-size, bf16)
    B2: L = ln(1+e); sp = max(x,0)+L        (softplus, bf16)
        nl = wn * sp                        (per-element negative loss, bf16)
        pos_sum = 0.75 * sum(wpx * (sp-x))
        cl = (u_eff < t_u) * nl             (candidate losses, chunked so the
                                             subsample is ready early)
  select:
    t_u: 2 Newton iterations on count(u_eff < t) targeting 10000 (u ~ U[0,1));
         traced between B1 and B2 so it overlaps the exp sweep
    k' = min(100*max(num_pos,1), 10000)
    t*: 5 rounds of 3-way threshold search on a 1/8 column subsample of cl,
        snapped to 0 when k' >= count(cl > 0)
    neg_sum = k'*t* + sum(relu(cl - t*))    (exact top-k' sum identity, 2nd-order
                                             insensitive to t* rank error)
    pos_loss = pos_sum/max(num_pos,1); neg_loss = neg_sum/max(num_pos,1)

Cross-partition (per-sample) reductions/broadcasts use a DVE-only 32x32
stream-transpose trick (no PE, no DMA round-trips). Big elementwise products run
on GPSIMD; weights/losses are stored bf16 (validated ~2e-5 relative error).
"""

import numpy as np

B, N = 64, 110592
P, S, J = 128, 8, 16          # partitions, samples/core, partitions/sample
FD = N // J                   # 6912 free columns
NCH = 8                       # stream chunks
CH = FD // NCH                # 864
NUM_NEG = 10000.0
LN4 = 1.3862944
BIGU = 4.0
NEWTON = 2
KPROBE = 3
ROUNDS = 5
SUBC = 864                    # stage-2 subsample columns (1/8 of FD)
SUBFRAC = SUBC / FD
HI0 = 3.0

_CACHE = {}


def _build():
    import concourse.bacc as bacc
    import concourse.bass as bass
    import concourse.tile as tile
    import concourse.mybir as mybir
    from contextlib import ExitStack

    dt = mybir.dt
    Alu = mybir.AluOpType
    Act = mybir.ActivationFunctionType

    nc = bacc.Bacc("TRN2", target_bir_lowering=False, debug=False)

    x_d = nc.dram_tensor("x", [S, N], dt.float32, kind="ExternalInput")
    t_d = nc.dram_tensor("t", [S, N], dt.float32, kind="ExternalInput")
    g_d = nc.dram_tensor("g", [S, N], dt.float32, kind="ExternalInput")
    u_d = nc.dram_tensor("u", [S, N], dt.float32, kind="ExternalInput")
    out_d = nc.dram_tensor("out", [S, 2], dt.float32, kind="ExternalOutput")

    xv = x_d.ap().rearrange("s (j f) -> (s j) f", j=J)
    tv = t_d.ap().rearrange("s (j f) -> (s j) f", j=J)
    gv = g_d.ap().rearrange("s (j f) -> (s j) f", j=J)
    uv = u_d.ap().rearrange("s (j f) -> (s j) f", j=J)

    f32 = dt.float32
    bf16 = dt.bfloat16

    with tile.TileContext(nc) as tc, ExitStack() as ctx:
        persist = ctx.enter_context(tc.tile_pool(name="persist", bufs=1))
        small = ctx.enter_context(tc.tile_pool(name="small", bufs=1))

        u_eff = persist.tile([P, FD], f32, tag="u_eff")
        x_full = persist.tile([P, FD], f32, tag="x_full")
        wn = persist.tile([P, FD], bf16, tag="wn")
        wpx = persist.tile([P, FD], bf16, tag="wpx")
        nl = persist.tile([P, FD], bf16, tag="nl")
        e_full = persist.tile([P, FD], bf16, tag="e_full")
        cl_sub = persist.tile([P, SUBC], bf16, tag="cl_sub")
        cl_rest = persist.tile([P, FD - SUBC], bf16, tag="cl_rest")
        scr = persist.tile([P, FD], bf16, tag="scr")

        np_cols = small.tile([P, NCH], f32, tag="np_cols")
        a12_cols = small.tile([P, NCH], f32, tag="a12_cols")
        cp_cols = small.tile([P, NCH], f32, tag="cp_cols")

        # ---- group-reduce helper (per-sample sums broadcast to the group) ----
        z = small.tile([P, 32], f32, tag="gr_z")
        zt = small.tile([P, 32], f32, tag="gr_zt")
        ra = small.tile([P, 1], f32, tag="gr_ra")
        rb = small.tile([P, 1], f32, tag="gr_rb")
        mm = small.tile([P, 32], f32, tag="gr_m")
        mt = small.tile([P, 32], f32, tag="gr_mt")

        def group_reduce(src_ap, ncols):
            nc.vector.memset(z, 0.0)
            nc.vector.tensor_copy(z[:, 0:ncols], src_ap)
            nc.vector.transpose(out=zt, in_=z)
            nc.vector.tensor_reduce(out=ra, in_=zt[:, 0:J],
                                    axis=mybir.AxisListType.X, op=Alu.add)
            nc.vector.tensor_reduce(out=rb, in_=zt[:, J:2 * J],
                                    axis=mybir.AxisListType.X, op=Alu.add)
            nc.vector.tensor_scalar(out=mm[:, 0:J], in0=zt[:, 0:J], scalar1=0.0,
                                    scalar2=ra, op0=Alu.mult, op1=Alu.add)
            nc.vector.tensor_scalar(out=mm[:, J:2 * J], in0=zt[:, 0:J], scalar1=0.0,
                                    scalar2=rb, op0=Alu.mult, op1=Alu.add)
            nc.vector.transpose(out=mt, in_=mm)
            return mt[:, 0:ncols]

        # ================= STREAM sweep A (sigmoid table) =================
        neg1 = small.tile([P, 1], f32, tag="neg1")
        nc.vector.memset(neg1, -1.0)
        lastA = None
        with tc.tile_pool(name="chunkA", bufs=2) as ca:
            for c in range(NCH):
                sl = slice(c * CH, (c + 1) * CH)
                tc_ = ca.tile([P, CH], f32, tag="tc")
                gc = ca.tile([P, CH], f32, tag="gc")
                uc = ca.tile([P, CH], f32, tag="uc")
                nc.sync.dma_start(out=x_full[:, sl], in_=xv[:, sl])
                nc.sync.dma_start(out=tc_, in_=tv[:, sl])
                nc.sync.dma_start(out=gc, in_=gv[:, sl])
                nc.sync.dma_start(out=uc, in_=uv[:, sl])

                p = ca.tile([P, CH], f32, tag="p")
                p2 = ca.tile([P, CH], bf16, tag="p2")
                pm2 = ca.tile([P, CH], bf16, tag="pm2")
                nc.scalar.activation(out=p, in_=x_full[:, sl], func=Act.Sigmoid)
                nc.scalar.activation(out=p2, in_=p, func=Act.Square)
                lastA = nc.scalar.activation(out=pm2, in_=p, func=Act.Square,
                                             bias=neg1)

                # neg-weight chain (bf16): rc=clip(5(p-.5),0,1); +1[x>0];
                # affine; *1[g==0]
                r1 = ca.tile([P, CH], bf16, tag="r1")
                nc.vector.tensor_scalar(out=r1, in0=p, scalar1=0.5, scalar2=5.0,
                                        op0=Alu.subtract, op1=Alu.mult)
                nc.vector.tensor_scalar(out=r1, in0=r1, scalar1=0.0, scalar2=1.0,
                                        op0=Alu.max, op1=Alu.min)
                nc.vector.scalar_tensor_tensor(out=r1, in0=x_full[:, sl],
                                               scalar=0.0, in1=r1,
                                               op0=Alu.is_gt, op1=Alu.add)
                nc.vector.tensor_scalar(out=r1, in0=r1, scalar1=0.125, scalar2=0.25,
                                        op0=Alu.mult, op1=Alu.add)
                gm = ca.tile([P, CH], bf16, tag="gm")
                nc.vector.tensor_scalar(out=gm, in0=gc, scalar1=0.0, scalar2=None,
                                        op0=Alu.is_equal)
                q2 = ca.tile([P, CH], bf16, tag="q2")
                nc.gpsimd.tensor_mul(q2, gm, r1)
                nc.gpsimd.tensor_mul(wn[:, sl], p2, q2)

                # pos factor (bf16): wq = t*(1+3*1[x<ln4]); wpx = pm2*wq
                wq = ca.tile([P, CH], bf16, tag="wq")
                nc.vector.tensor_scalar(out=wq, in0=x_full[:, sl], scalar1=LN4,
                                        scalar2=-3.0, op0=Alu.is_ge, op1=Alu.mult)
                nc.vector.scalar_tensor_tensor(out=wq, in0=wq, scalar=4.0, in1=tc_,
                                               op0=Alu.add, op1=Alu.mult)
                nc.gpsimd.tensor_mul(wpx[:, sl], pm2, wq)

                # u_eff = 4*t + u  (f32: u's 2^-23 grid must survive)
                nc.vector.scalar_tensor_tensor(out=u_eff[:, sl], in0=tc_,
                                               scalar=BIGU, in1=uc, op0=Alu.mult,
                                               op1=Alu.add)
                # num_pos partial
                nc.vector.tensor_scalar(out=uc, in0=tc_, scalar1=1.0, scalar2=None,
                                        op0=Alu.mult, op1=Alu.add,
                                        accum_out=np_cols[:, c:c + 1])

        # ================= STREAM sweep B1 (exp table) =================
        lastB1 = None
        with tc.tile_pool(name="chunkB1", bufs=2) as cb1:
            for c in range(NCH):
                sl = slice(c * CH, (c + 1) * CH)
                ax = cb1.tile([P, CH], f32, tag="ax")
                ins_abs = nc.scalar.activation(out=ax, in_=x_full[:, sl],
                                               func=Act.Abs)
                tile.add_dep_helper(ins_abs.ins, lastA.ins, sync=True,
                                    reason="act-fence-A-B1")
                lastB1 = nc.scalar.activation(out=e_full[:, sl], in_=ax,
                                              func=Act.Exp, scale=-1.0)

        # ====== stage 1 Newton (overlaps B1 on the scheduler) ======
        npsum = small.tile([P, 1], f32, tag="npsum")
        nc.vector.tensor_reduce(out=npsum, in_=np_cols, axis=mybir.AxisListType.X,
                                op=Alu.add)
        np_g = group_reduce(npsum, 1)
        np128 = small.tile([P, 1], f32, tag="np128")
        nc.vector.tensor_copy(np128, np_g)
        nneg = small.tile([P, 1], f32, tag="nneg")
        invn = small.tile([P, 1], f32, tag="invn")
        nc.vector.tensor_scalar(out=nneg, in0=np128, scalar1=-1.0,
                                scalar2=float(N), op0=Alu.mult, op1=Alu.add)
        nc.vector.reciprocal(out=invn, in_=nneg)

        t128 = small.tile([P, 1], f32, tag="t128")
        acc1 = small.tile([P, 1], f32, tag="acc1")
        diff = small.tile([P, 1], f32, tag="diff")
        nc.vector.tensor_scalar(out=t128, in0=invn, scalar1=NUM_NEG,
                                scalar2=None, op0=Alu.mult)
        for it in range(NEWTON):
            nc.vector.tensor_scalar(out=scr, in0=u_eff, scalar1=t128,
                                    scalar2=None, op0=Alu.is_lt, op1=Alu.add,
                                    accum_out=acc1)
            c128 = group_reduce(acc1, 1)
            nc.vector.tensor_scalar(out=diff, in0=c128, scalar1=-1.0,
                                    scalar2=NUM_NEG, op0=Alu.mult, op1=Alu.add)
            nc.vector.tensor_mul(diff, diff, invn)
            nc.vector.tensor_add(t128, t128, diff)

        # ================= STREAM sweep B2 (ln table) =================
        first_ln = True
        with tc.tile_pool(name="chunkB2", bufs=3) as cb2:
            for c in range(NCH):
                sl = slice(c * CH, (c + 1) * CH)
                L = cb2.tile([P, CH], bf16, tag="L")
                ins_ln = nc.scalar.activation(out=L, in_=e_full[:, sl],
                                              func=Act.Ln, bias=1.0)
                if first_ln:
                    tile.add_dep_helper(ins_ln.ins, lastB1.ins, sync=True,
                                        reason="act-fence-B1-B2")
                    first_ln = False
                sp = cb2.tile([P, CH], bf16, tag="sp")
                nc.vector.scalar_tensor_tensor(out=sp, in0=x_full[:, sl],
                                               scalar=0.0, in1=L, op0=Alu.max,
                                               op1=Alu.add)
                rxm = cb2.tile([P, CH], bf16, tag="rxm")
                nc.scalar.activation(out=rxm, in_=x_full[:, sl], func=Act.Relu,
                                     scale=-1.0)
                spm = cb2.tile([P, CH], bf16, tag="spm")
                nc.gpsimd.tensor_add(spm, rxm, L)
                nc.gpsimd.tensor_mul(nl[:, sl], wn[:, sl], sp)
                ttr = cb2.tile([P, CH], bf16, tag="ttr")
                nc.vector.scalar_tensor_tensor(out=ttr, in0=wpx[:, sl], scalar=1.0,
                                               in1=spm, op0=Alu.mult, op1=Alu.mult,
                                               accum_out=a12_cols[:, c:c + 1])
                # candidate losses + c_pos partials (needs t128 from Newton)
                cl_c = cl_sub if c == 0 else cl_rest[:, (c - 1) * CH:c * CH]
                nc.vector.scalar_tensor_tensor(out=cl_c, in0=u_eff[:, sl],
                                               scalar=t128, in1=nl[:, sl],
                                               op0=Alu.is_lt, op1=Alu.mult)
                nc.vector.tensor_scalar(out=scr[:, sl], in0=cl_c, scalar1=0.0,
                                        scalar2=None, op0=Alu.is_gt, op1=Alu.add,
                                        accum_out=cp_cols[:, c:c + 1])

        # dedicated round-scratch (so stage-2 doesn't serialize behind B2)
        z2 = small.tile([P, 32], f32, tag="r_z")
        zt2 = small.tile([P, 32], f32, tag="r_zt")
        ra2 = small.tile([P, 1], f32, tag="r_ra")
        rb2 = small.tile([P, 1], f32, tag="r_rb")
        mm2 = small.tile([P, 32], f32, tag="r_m")
        mt2 = small.tile([P, 32], f32, tag="r_mt")
        scr2 = small.tile([P, SUBC], bf16, tag="r_scr")

        def group_reduce2(src_ap, ncols):
            nc.vector.memset(z2, 0.0)
            nc.vector.tensor_copy(z2[:, 0:ncols], src_ap)
            nc.vector.transpose(out=zt2, in_=z2)
            nc.vector.tensor_reduce(out=ra2, in_=zt2[:, 0:J],
                                    axis=mybir.AxisListType.X, op=Alu.add)
            nc.vector.tensor_reduce(out=rb2, in_=zt2[:, J:2 * J],
                                    axis=mybir.AxisListType.X, op=Alu.add)
            nc.vector.tensor_scalar(out=mm2[:, 0:J], in0=zt2[:, 0:J], scalar1=0.0,
                                    scalar2=ra2, op0=Alu.mult, op1=Alu.add)
            nc.vector.tensor_scalar(out=mm2[:, J:2 * J], in0=zt2[:, 0:J],
                                    scalar1=0.0, scalar2=rb2, op0=Alu.mult,
                                    op1=Alu.add)
            nc.vector.transpose(out=mt2, in_=mm2)
            return mt2[:, 0:ncols]

        # k' = min(100*max(num_pos,1), 10000)
        kk = small.tile([P, 1], f32, tag="kk")
        nc.vector.tensor_scalar(out=kk, in0=np128, scalar1=1.0, scalar2=100.0,
                                op0=Alu.max, op1=Alu.mult)
        nc.vector.tensor_scalar(out=kk, in0=kk, scalar1=NUM_NEG, scalar2=None,
                                op0=Alu.min)

        # ================= stage 2: K-way threshold search =================
        kt = small.tile([P, 1], f32, tag="kt")
        nc.vector.tensor_scalar(out=kt, in0=kk, scalar1=SUBFRAC, scalar2=None,
                                op0=Alu.mult)
        lo = small.tile([P, 1], f32, tag="lo")
        hi = small.tile([P, 1], f32, tag="hi")
        nc.vector.memset(lo, 0.0)
        nc.vector.memset(hi, HI0)
        fracs = small.tile([P, KPROBE], f32, tag="fracs")
        for i in range(KPROBE):
            nc.vector.memset(fracs[:, i:i + 1], (i + 1.0) / (KPROBE + 1.0))
        thrK = small.tile([P, KPROBE], f32, tag="thrK")
        accK = small.tile([P, KPROBE], f32, tag="accK")
        gtm = small.tile([P, KPROBE], f32, tag="gtm")
        hic = small.tile([P, KPROBE], f32, tag="hic")
        width = small.tile([P, 1], f32, tag="width")
        lomax = small.tile([P, 1], f32, tag="lomax")
        himin = small.tile([P, 1], f32, tag="himin")

        for r in range(ROUNDS):
            nc.vector.tensor_sub(width, hi, lo)
            nc.vector.tensor_scalar(out=thrK, in0=fracs, scalar1=width,
                                    scalar2=lo, op0=Alu.mult, op1=Alu.add)
            for i in range(KPROBE):
                nc.vector.tensor_scalar(out=scr2, in0=cl_sub,
                                        scalar1=thrK[:, i:i + 1], scalar2=None,
                                        op0=Alu.is_gt, op1=Alu.add,
                                        accum_out=accK[:, i:i + 1])
            csK = group_reduce2(accK, KPROBE)
            nc.vector.tensor_scalar(out=gtm, in0=csK, scalar1=kt, scalar2=None,
                                    op0=Alu.is_gt)
            nc.vector.tensor_mul(hic, thrK, gtm)
            nc.vector.tensor_reduce(out=lomax, in_=hic,
                                    axis=mybir.AxisListType.X, op=Alu.max)
            nc.vector.tensor_tensor(out=lo, in0=lo, in1=lomax, op=Alu.max)
            nc.vector.scalar_tensor_tensor(out=hic, in0=gtm, scalar=1e9,
                                           in1=thrK, op0=Alu.mult, op1=Alu.add)
            nc.vector.tensor_reduce(out=himin, in_=hic,
                                    axis=mybir.AxisListType.X, op=Alu.min)
            nc.vector.tensor_tensor(out=hi, in0=hi, in1=himin, op=Alu.min)

        a12s = small.tile([P, 1], f32, tag="a12s")
        cps = small.tile([P, 1], f32, tag="cps")
        nc.vector.tensor_reduce(out=a12s, in_=a12_cols, axis=mybir.AxisListType.X,
                                op=Alu.add)
        nc.vector.tensor_reduce(out=cps, in_=cp_cols, axis=mybir.AxisListType.X,
                                op=Alu.add)
        pair = small.tile([P, 2], f32, tag="pair")
        nc.vector.tensor_copy(pair[:, 0:1], a12s)
        nc.vector.tensor_copy(pair[:, 1:2], cps)
        gr = group_reduce(pair, 2)
        pos128 = small.tile([P, 1], f32, tag="pos128")
        cpos = small.tile([P, 1], f32, tag="cpos")
        nc.vector.tensor_scalar(out=pos128, in0=gr[:, 0:1], scalar1=0.75,
                                scalar2=None, op0=Alu.mult)
        nc.vector.tensor_copy(cpos, gr[:, 1:2])

        tstar = small.tile([P, 1], f32, tag="tstar")
        nc.vector.tensor_add(tstar, lo, hi)
        nc.vector.tensor_scalar(out=tstar, in0=tstar, scalar1=0.5, scalar2=None,
                                op0=Alu.mult)
        klt = small.tile([P, 1], f32, tag="klt")
        nc.vector.tensor_tensor(out=klt, in0=kk, in1=cpos, op=Alu.is_lt)
        nc.vector.tensor_mul(tstar, tstar, klt)

        # neg_sum = k'*t* + sum(relu(cl - t*))
        nbias = small.tile([P, 1], f32, tag="nbias")
        nc.vector.tensor_scalar(out=nbias, in0=tstar, scalar1=-1.0, scalar2=None,
                                op0=Alu.mult)
        racc = small.tile([P, 1], f32, tag="racc")
        racc2 = small.tile([P, 1], f32, tag="racc2")
        relsub = small.tile([P, SUBC], bf16, tag="relsub")
        nc.vector.tensor_scalar(out=relsub, in0=cl_sub, scalar1=nbias, scalar2=0.0,
                                op0=Alu.add, op1=Alu.max)
        nc.vector.tensor_scalar(out=scr2, in0=relsub, scalar1=1.0, scalar2=None,
                                op0=Alu.mult, op1=Alu.add, accum_out=racc)
        nc.scalar.activation(out=scr[:, SUBC:FD], in_=cl_rest, func=Act.Relu,
                             bias=nbias, accum_out=racc2)
        nc.vector.tensor_add(racc, racc, racc2)
        rsum_g = group_reduce(racc, 1)
        negsum = small.tile([P, 1], f32, tag="negsum")
        nc.vector.tensor_mul(negsum, kk, tstar)
        nc.vector.tensor_add(negsum, negsum, rsum_g)

        # final losses + output
        denom = small.tile([P, 1], f32, tag="denom")
        invd = small.tile([P, 1], f32, tag="invd")
        nc.vector.tensor_scalar(out=denom, in0=np128, scalar1=1.0, scalar2=None,
                                op0=Alu.max)
        nc.vector.reciprocal(out=invd, in_=denom)
        out128 = small.tile([P, 2], f32, tag="out128")
        nc.vector.tensor_mul(out128[:, 0:1], pos128, invd)
        nc.vector.tensor_mul(out128[:, 1:2], negsum, invd)
        pstride = out128.ap[0][0]
        src = bass.AP(tensor=out128.tensor, offset=out128.offset,
                      ap=[[J * pstride, S], [1, 2]])
        nc.sync.dma_start(out=out_d.ap(), in_=src)

    nc.compile()
    return nc


def _get_nc():
    if "nc" not in _CACHE:
        _CACHE["nc"] = _build()
    return _CACHE["nc"]


def kernel(pred, target, mask_ignore, neg_rand):
    from concourse.bass_utils import run_bass_kernel_spmd

    nc = _get_nc()
    pred = np.ascontiguousarray(np.asarray(pred, dtype=np.float32).reshape(B, N))
    target = np.ascontiguousarray(np.asarray(target, dtype=np.float32).reshape(B, N))
    mask_ignore = np.ascontiguousarray(
        np.asarray(mask_ignore, dtype=np.float32).reshape(B, N))
    neg_rand = np.ascontiguousarray(
        np.asarray(neg_rand, dtype=np.float32).reshape(B, N))

    n_cores = B // S
    in_maps = []
    for c in range(n_cores):
        sl = slice(c * S, (c + 1) * S)
        in_maps.append({
            "x": pred[sl], "t": target[sl], "g": mask_ignore[sl], "u": neg_rand[sl],
        })
    res = run_bass_kernel_spmd(nc, in_maps, core_ids=list(range(n_cores)))
    outs = np.stack([r["out"] for r in res.results])  # [cores, S, 2]
    pos = np.float32(outs[:, :, 0].sum(dtype=np.float64) / B)
    neg = np.float32(outs[:, :, 1].sum(dtype=np.float64) / B)
    return pos, neg


# revision 24
# speedup vs baseline: 1.0164x; 1.0164x over previous
"""Trainium2 Bass kernel for nn_DetectionLoss (focal BCE + online hard negative mining).

Contract: kernel(**inputs) takes FULL inputs (pred/target/mask_ignore/neg_rand,
each [64, 110592, 1] f32) and returns the full output (cls_pos_loss, cls_neg_loss)
as two f32 scalars, matching the jax reference.

Sharding: pure data parallel over the batch dim — 8 samples per NeuronCore,
8 cores. Each core computes per-sample (pos_loss, neg_loss); the host sums the
64 pairs and divides by B (the all-reduce of two scalars).

Device algorithm per sample (layout: sample = 16 partitions x 6912 cols;
8 samples stacked -> [128, 6912] per core):
  stream (3 ACT-table sweeps, fenced so each table loads exactly once):
    A:  p = sigmoid(x)
        wn  = p^2 * (0.25 + 0.125*(clip(5(p-.5),0,1) + 1[x>0])) * 1[g==0]
        wpx = (1-p)^2 * t * (1 + 3*1[x<ln4])
        u_eff = u + 4t;  num_pos = sum(t)
    B1: e = exp(-|x|)                       (staged full-size, bf16)
    B2: L = ln(1+e); sp = max(x,0)+L        (softplus, bf16)
        nl = wn * sp                        (per-element negative loss, bf16)
        pos_sum = 0.75 * sum(wpx * (sp-x))
        cl = (u_eff < t_u) * nl             (candidate losses, chunked so the
                                             subsample is ready early)
  select:
    t_u: 2 Newton iterations on count(u_eff < t) targeting 10000 (u ~ U[0,1));
         traced between B1 and B2 so it overlaps the exp sweep
    k' = min(100*max(num_pos,1), 10000)
    t*: 5 rounds of 3-way threshold search on a 1/8 column subsample of cl,
        snapped to 0 when k' >= count(cl > 0)
    neg_sum = k'*t* + sum(relu(cl - t*))    (exact top-k' sum identity, 2nd-order
                                             insensitive to t* rank error)
    pos_loss = pos_sum/max(num_pos,1); neg_loss = neg_sum/max(num_pos,1)

Cross-partition (per-sample) reductions/broadcasts use a DVE-only 32x32
stream-transpose trick (no PE, no DMA round-trips). Big elementwise products run
on GPSIMD; weights/losses are stored bf16 (validated ~2e-5 relative error).
"""

import numpy as np

B, N = 64, 110592
P, S, J = 128, 8, 16          # partitions, samples/core, partitions/sample
FD = N // J                   # 6912 free columns
NCH = 8                       # stream chunks
CH = FD // NCH                # 864
NUM_NEG = 10000.0
LN4 = 1.3862944
BIGU = 4.0
NEWTON = 2
KPROBE = 3
ROUNDS = 5
SUBC = 864                    # stage-2 subsample columns (1/8 of FD)
SUBFRAC = SUBC / FD
HI0 = 3.0

_CACHE = {}


def _build():
    import concourse.bacc as bacc
    import concourse.bass as bass
    import concourse.tile as tile
    import concourse.mybir as mybir
    from contextlib import ExitStack

    dt = mybir.dt
    Alu = mybir.AluOpType
    Act = mybir.ActivationFunctionType

    nc = bacc.Bacc("TRN2", target_bir_lowering=False, debug=False)

    x_d = nc.dram_tensor("x", [S, N], dt.float32, kind="ExternalInput")
    t_d = nc.dram_tensor("t", [S, N], dt.float32, kind="ExternalInput")
    g_d = nc.dram_tensor("g", [S, N], dt.float32, kind="ExternalInput")
    u_d = nc.dram_tensor("u", [S, N], dt.float32, kind="ExternalInput")
    out_d = nc.dram_tensor("out", [S, 2], dt.float32, kind="ExternalOutput")

    xv = x_d.ap().rearrange("s (j f) -> (s j) f", j=J)
    tv = t_d.ap().rearrange("s (j f) -> (s j) f", j=J)
    gv = g_d.ap().rearrange("s (j f) -> (s j) f", j=J)
    uv = u_d.ap().rearrange("s (j f) -> (s j) f", j=J)

    f32 = dt.float32
    bf16 = dt.bfloat16

    with tile.TileContext(nc) as tc, ExitStack() as ctx:
        persist = ctx.enter_context(tc.tile_pool(name="persist", bufs=1))
        small = ctx.enter_context(tc.tile_pool(name="small", bufs=1))

        u_eff = persist.tile([P, FD], f32, tag="u_eff")
        x_full = persist.tile([P, FD], f32, tag="x_full")
        wn = persist.tile([P, FD], bf16, tag="wn")
        wpx = persist.tile([P, FD], bf16, tag="wpx")
        nl = persist.tile([P, FD], bf16, tag="nl")
        e_full = persist.tile([P, FD], bf16, tag="e_full")
        cl_sub = persist.tile([P, SUBC], bf16, tag="cl_sub")
        cl_rest = persist.tile([P, FD - SUBC], bf16, tag="cl_rest")
        scr = persist.tile([P, FD], bf16, tag="scr")

        np_cols = small.tile([P, NCH], f32, tag="np_cols")
        a12_cols = small.tile([P, NCH], f32, tag="a12_cols")
        cp_cols = small.tile([P, NCH], f32, tag="cp_cols")

        # ---- group-reduce helper (per-sample sums broadcast to the group) ----
        z = small.tile([P, 32], f32, tag="gr_z")
        zt = small.tile([P, 32], f32, tag="gr_zt")
        ra = small.tile([P, 1], f32, tag="gr_ra")
        rb = small.tile([P, 1], f32, tag="gr_rb")
        mm = small.tile([P, 32], f32, tag="gr_m")
        mt = small.tile([P, 32], f32, tag="gr_mt")

        def group_reduce(src_ap, ncols):
            nc.vector.memset(z, 0.0)
            nc.vector.tensor_copy(z[:, 0:ncols], src_ap)
            nc.vector.transpose(out=zt, in_=z)
            nc.vector.tensor_reduce(out=ra, in_=zt[:, 0:J],
                                    axis=mybir.AxisListType.X, op=Alu.add)
            nc.vector.tensor_reduce(out=rb, in_=zt[:, J:2 * J],
                                    axis=mybir.AxisListType.X, op=Alu.add)
            nc.vector.tensor_scalar(out=mm[:, 0:J], in0=zt[:, 0:J], scalar1=0.0,
                                    scalar2=ra, op0=Alu.mult, op1=Alu.add)
            nc.vector.tensor_scalar(out=mm[:, J:2 * J], in0=zt[:, 0:J], scalar1=0.0,
                                    scalar2=rb, op0=Alu.mult, op1=Alu.add)
            nc.vector.transpose(out=mt, in_=mm)
            return mt[:, 0:ncols]

        # ================= STREAM sweep A (sigmoid table) =================
        neg1 = small.tile([P, 1], f32, tag="neg1")
        nc.vector.memset(neg1, -1.0)
        lastA = None
        with tc.tile_pool(name="chunkA", bufs=2) as ca:
            for c in range(NCH):
                sl = slice(c * CH, (c + 1) * CH)
                tc_ = ca.tile([P, CH], f32, tag="tc")
                gc = ca.tile([P, CH], f32, tag="gc")
                uc = ca.tile([P, CH], f32, tag="uc")
                nc.sync.dma_start(out=x_full[:, sl], in_=xv[:, sl])
                nc.sync.dma_start(out=tc_, in_=tv[:, sl])
                nc.sync.dma_start(out=gc, in_=gv[:, sl])
                nc.sync.dma_start(out=uc, in_=uv[:, sl])

                p = ca.tile([P, CH], f32, tag="p")
                p2 = ca.tile([P, CH], bf16, tag="p2")
                pm2 = ca.tile([P, CH], bf16, tag="pm2")
                nc.scalar.activation(out=p, in_=x_full[:, sl], func=Act.Sigmoid)
                nc.scalar.activation(out=p2, in_=p, func=Act.Square)
                lastA = nc.scalar.activation(out=pm2, in_=p, func=Act.Square,
                                             bias=neg1)

                # neg-weight chain (bf16): rc=clip(5(p-.5),0,1); +1[x>0];
                # affine; *1[g==0]
                r1 = ca.tile([P, CH], bf16, tag="r1")
                nc.vector.tensor_scalar(out=r1, in0=p, scalar1=0.5, scalar2=5.0,
                                        op0=Alu.subtract, op1=Alu.mult)
                nc.vector.tensor_scalar(out=r1, in0=r1, scalar1=0.0, scalar2=1.0,
                                        op0=Alu.max, op1=Alu.min)
                nc.vector.scalar_tensor_tensor(out=r1, in0=x_full[:, sl],
                                               scalar=0.0, in1=r1,
                                               op0=Alu.is_gt, op1=Alu.add)
                nc.vector.tensor_scalar(out=r1, in0=r1, scalar1=0.125, scalar2=0.25,
                                        op0=Alu.mult, op1=Alu.add)
                gm = ca.tile([P, CH], bf16, tag="gm")
                nc.vector.tensor_scalar(out=gm, in0=gc, scalar1=0.0, scalar2=None,
                                        op0=Alu.is_equal)
                q2 = ca.tile([P, CH], bf16, tag="q2")
                nc.gpsimd.tensor_mul(q2, gm, r1)
                nc.gpsimd.tensor_mul(wn[:, sl], p2, q2)

                # pos factor (bf16): wq = t*(1+3*1[x<ln4]); wpx = pm2*wq
                wq = ca.tile([P, CH], bf16, tag="wq")
                nc.vector.tensor_scalar(out=wq, in0=x_full[:, sl], scalar1=LN4,
                                        scalar2=-3.0, op0=Alu.is_ge, op1=Alu.mult)
                nc.vector.scalar_tensor_tensor(out=wq, in0=wq, scalar=4.0, in1=tc_,
                                               op0=Alu.add, op1=Alu.mult)
                nc.gpsimd.tensor_mul(wpx[:, sl], pm2, wq)

                # u_eff = 4*t + u  (f32: u's 2^-23 grid must survive)
                nc.vector.scalar_tensor_tensor(out=u_eff[:, sl], in0=tc_,
                                               scalar=BIGU, in1=uc, op0=Alu.mult,
                                               op1=Alu.add)
                # num_pos partial
                nc.vector.tensor_scalar(out=uc, in0=tc_, scalar1=1.0, scalar2=None,
                                        op0=Alu.mult, op1=Alu.add,
                                        accum_out=np_cols[:, c:c + 1])

        # ================= STREAM sweep B1 (exp table) =================
        lastB1 = None
        with tc.tile_pool(name="chunkB1", bufs=2) as cb1:
            for c in range(NCH):
                sl = slice(c * CH, (c + 1) * CH)
                ax = cb1.tile([P, CH], f32, tag="ax")
                ins_abs = nc.scalar.activation(out=ax, in_=x_full[:, sl],
                                               func=Act.Abs)
                tile.add_dep_helper(ins_abs.ins, lastA.ins, sync=True,
                                    reason="act-fence-A-B1")
                lastB1 = nc.scalar.activation(out=e_full[:, sl], in_=ax,
                                              func=Act.Exp, scale=-1.0)

        # ====== stage 1 Newton (overlaps B1 on the scheduler) ======
        npsum = small.tile([P, 1], f32, tag="npsum")
        nc.vector.tensor_reduce(out=npsum, in_=np_cols, axis=mybir.AxisListType.X,
                                op=Alu.add)
        np_g = group_reduce(npsum, 1)
        np128 = small.tile([P, 1], f32, tag="np128")
        nc.vector.tensor_copy(np128, np_g)
        nneg = small.tile([P, 1], f32, tag="nneg")
        invn = small.tile([P, 1], f32, tag="invn")
        nc.vector.tensor_scalar(out=nneg, in0=np128, scalar1=-1.0,
                                scalar2=float(N), op0=Alu.mult, op1=Alu.add)
        nc.vector.reciprocal(out=invn, in_=nneg)

        t128 = small.tile([P, 1], f32, tag="t128")
        acc1 = small.tile([P, 1], f32, tag="acc1")
        diff = small.tile([P, 1], f32, tag="diff")
        nc.vector.tensor_scalar(out=t128, in0=invn, scalar1=NUM_NEG,
                                scalar2=None, op0=Alu.mult)
        for it in range(NEWTON):
            nc.vector.tensor_scalar(out=scr, in0=u_eff, scalar1=t128,
                                    scalar2=None, op0=Alu.is_lt, op1=Alu.add,
                                    accum_out=acc1)
            c128 = group_reduce(acc1, 1)
            nc.vector.tensor_scalar(out=diff, in0=c128, scalar1=-1.0,
                                    scalar2=NUM_NEG, op0=Alu.mult, op1=Alu.add)
            nc.vector.tensor_mul(diff, diff, invn)
            nc.vector.tensor_add(t128, t128, diff)

        # ================= STREAM sweep B2 (ln table) =================
        first_ln = True
        with tc.tile_pool(name="chunkB2", bufs=3) as cb2:
            for c in range(NCH):
                sl = slice(c * CH, (c + 1) * CH)
                L = cb2.tile([P, CH], bf16, tag="L")
                ins_ln = nc.scalar.activation(out=L, in_=e_full[:, sl],
                                              func=Act.Ln, bias=1.0)
                if first_ln:
                    tile.add_dep_helper(ins_ln.ins, lastB1.ins, sync=True,
                                        reason="act-fence-B1-B2")
                    first_ln = False
                sp = cb2.tile([P, CH], bf16, tag="sp")
                nc.vector.scalar_tensor_tensor(out=sp, in0=x_full[:, sl],
                                               scalar=0.0, in1=L, op0=Alu.max,
                                               op1=Alu.add)
                rxm = cb2.tile([P, CH], bf16, tag="rxm")
                nc.scalar.activation(out=rxm, in_=x_full[:, sl], func=Act.Relu,
                                     scale=-1.0)
                spm = cb2.tile([P, CH], bf16, tag="spm")
                nc.gpsimd.tensor_add(spm, rxm, L)
                nc.gpsimd.tensor_mul(nl[:, sl], wn[:, sl], sp)
                ttr = cb2.tile([P, CH], bf16, tag="ttr")
                nc.vector.scalar_tensor_tensor(out=ttr, in0=wpx[:, sl], scalar=1.0,
                                               in1=spm, op0=Alu.mult, op1=Alu.mult,
                                               accum_out=a12_cols[:, c:c + 1])
                # candidate losses + c_pos partials (needs t128 from Newton)
                cl_c = cl_sub if c == 0 else cl_rest[:, (c - 1) * CH:c * CH]
                nc.vector.scalar_tensor_tensor(out=cl_c, in0=u_eff[:, sl],
                                               scalar=t128, in1=nl[:, sl],
                                               op0=Alu.is_lt, op1=Alu.mult)
                nc.vector.tensor_scalar(out=scr[:, sl], in0=cl_c, scalar1=0.0,
                                        scalar2=None, op0=Alu.is_gt, op1=Alu.add,
                                        accum_out=cp_cols[:, c:c + 1])

        # dedicated round-scratch (so stage-2 doesn't serialize behind B2)
        z2 = small.tile([P, 32], f32, tag="r_z")
        zt2 = small.tile([P, 32], f32, tag="r_zt")
        ra2 = small.tile([P, 1], f32, tag="r_ra")
        rb2 = small.tile([P, 1], f32, tag="r_rb")
        mm2 = small.tile([P, 32], f32, tag="r_m")
        mt2 = small.tile([P, 32], f32, tag="r_mt")
        scr2 = small.tile([P, SUBC], bf16, tag="r_scr")

        def group_reduce2(src_ap, ncols):
            nc.vector.memset(z2, 0.0)
            nc.vector.tensor_copy(z2[:, 0:ncols], src_ap)
            nc.vector.transpose(out=zt2, in_=z2)
            nc.vector.tensor_reduce(out=ra2, in_=zt2[:, 0:J],
                                    axis=mybir.AxisListType.X, op=Alu.add)
            nc.vector.tensor_reduce(out=rb2, in_=zt2[:, J:2 * J],
                                    axis=mybir.AxisListType.X, op=Alu.add)
            nc.vector.tensor_scalar(out=mm2[:, 0:J], in0=zt2[:, 0:J], scalar1=0.0,
                                    scalar2=ra2, op0=Alu.mult, op1=Alu.add)
            nc.vector.tensor_scalar(out=mm2[:, J:2 * J], in0=zt2[:, 0:J],
                                    scalar1=0.0, scalar2=rb2, op0=Alu.mult,
                                    op1=Alu.add)
            nc.vector.transpose(out=mt2, in_=mm2)
            return mt2[:, 0:ncols]

        # k' = min(100*max(num_pos,1), 10000)
        kk = small.tile([P, 1], f32, tag="kk")
        nc.vector.tensor_scalar(out=kk, in0=np128, scalar1=1.0, scalar2=100.0,
                                op0=Alu.max, op1=Alu.mult)
        nc.vector.tensor_scalar(out=kk, in0=kk, scalar1=NUM_NEG, scalar2=None,
                                op0=Alu.min)

        # ================= stage 2: K-way threshold search =================
        kt = small.tile([P, 1], f32, tag="kt")
        nc.vector.tensor_scalar(out=kt, in0=kk, scalar1=SUBFRAC, scalar2=None,
                                op0=Alu.mult)
        lo = small.tile([P, 1], f32, tag="lo")
        hi = small.tile([P, 1], f32, tag="hi")
        nc.vector.memset(lo, 0.0)
        nc.vector.memset(hi, HI0)
        fracs = small.tile([P, KPROBE], f32, tag="fracs")
        for i in range(KPROBE):
            nc.vector.memset(fracs[:, i:i + 1], (i + 1.0) / (KPROBE + 1.0))
        thrK = small.tile([P, KPROBE], f32, tag="thrK")
        accK = small.tile([P, KPROBE], f32, tag="accK")
        gtm = small.tile([P, KPROBE], f32, tag="gtm")
        hic = small.tile([P, KPROBE], f32, tag="hic")
        width = small.tile([P, 1], f32, tag="width")
        lomax = small.tile([P, 1], f32, tag="lomax")
        himin = small.tile([P, 1], f32, tag="himin")

        for r in range(ROUNDS):
            nc.vector.tensor_sub(width, hi, lo)
            nc.vector.tensor_scalar(out=thrK, in0=fracs, scalar1=width,
                                    scalar2=lo, op0=Alu.mult, op1=Alu.add)
            for i in range(KPROBE):
                nc.vector.tensor_scalar(out=scr2, in0=cl_sub,
                                        scalar1=thrK[:, i:i + 1], scalar2=None,
                                        op0=Alu.is_gt, op1=Alu.add,
                                        accum_out=accK[:, i:i + 1])
            csK = group_reduce2(accK, KPROBE)
            nc.vector.tensor_scalar(out=gtm, in0=csK, scalar1=kt, scalar2=None,
                                    op0=Alu.is_gt)
            nc.vector.tensor_mul(hic, thrK, gtm)
            nc.vector.tensor_reduce(out=lomax, in_=hic,
                                    axis=mybir.AxisListType.X, op=Alu.max)
            nc.vector.tensor_tensor(out=lo, in0=lo, in1=lomax, op=Alu.max)
            nc.vector.scalar_tensor_tensor(out=hic, in0=gtm, scalar=1e9,
                                           in1=thrK, op0=Alu.mult, op1=Alu.add)
            nc.vector.tensor_reduce(out=himin, in_=hic,
                                    axis=mybir.AxisListType.X, op=Alu.min)
            nc.vector.tensor_tensor(out=hi, in0=hi, in1=himin, op=Alu.min)

        a12s = small.tile([P, 1], f32, tag="a12s")
        cps = small.tile([P, 1], f32, tag="cps")
        nc.vector.tensor_reduce(out=a12s, in_=a12_cols, axis=mybir.AxisListType.X,
                                op=Alu.add)
        nc.vector.tensor_reduce(out=cps, in_=cp_cols, axis=mybir.AxisListType.X,
                                op=Alu.add)
        pair = small.tile([P, 2], f32, tag="pair")
        nc.vector.tensor_copy(pair[:, 0:1], a12s)
        nc.vector.tensor_copy(pair[:, 1:2], cps)
        gr = group_reduce(pair, 2)
        pos128 = small.tile([P, 1], f32, tag="pos128")
        cpos = small.tile([P, 1], f32, tag="cpos")
        nc.vector.tensor_scalar(out=pos128, in0=gr[:, 0:1], scalar1=0.75,
                                scalar2=None, op0=Alu.mult)
        nc.vector.tensor_copy(cpos, gr[:, 1:2])

        tstar = small.tile([P, 1], f32, tag="tstar")
        nc.vector.tensor_add(tstar, lo, hi)
        nc.vector.tensor_scalar(out=tstar, in0=tstar, scalar1=0.5, scalar2=None,
                                op0=Alu.mult)
        klt = small.tile([P, 1], f32, tag="klt")
        nc.vector.tensor_tensor(out=klt, in0=kk, in1=cpos, op=Alu.is_lt)
        nc.vector.tensor_mul(tstar, tstar, klt)

        # neg_sum = k'*t* + sum(relu(cl - t*))
        nbias = small.tile([P, 1], f32, tag="nbias")
        nc.vector.tensor_scalar(out=nbias, in0=tstar, scalar1=-1.0, scalar2=None,
                                op0=Alu.mult)
        racc = small.tile([P, 1], f32, tag="racc")
        racc2 = small.tile([P, 1], f32, tag="racc2")
        racc3 = small.tile([P, 1], f32, tag="racc3")
        relsub = small.tile([P, SUBC], bf16, tag="relsub")
        nc.vector.tensor_scalar(out=relsub, in0=cl_sub, scalar1=nbias, scalar2=0.0,
                                op0=Alu.add, op1=Alu.max)
        nc.vector.tensor_scalar(out=scr2, in0=relsub, scalar1=1.0, scalar2=None,
                                op0=Alu.mult, op1=Alu.add, accum_out=racc)
        # split cl_rest's relu-sum: DVE takes the first RD cols in parallel
        # with ACT's remaining cols (disjoint scr regions so they overlap)
        RD = 2592
        relrd = small.tile([P, RD], bf16, tag="relrd")
        nc.vector.tensor_scalar(out=relrd, in0=cl_rest[:, 0:RD], scalar1=nbias,
                                scalar2=0.0, op0=Alu.add, op1=Alu.max)
        nc.vector.tensor_scalar(out=scr[:, 0:RD], in0=relrd, scalar1=1.0,
                                scalar2=None, op0=Alu.mult, op1=Alu.add,
                                accum_out=racc3)
        nc.scalar.activation(out=scr[:, SUBC + RD:FD], in_=cl_rest[:, RD:],
                             func=Act.Relu, bias=nbias, accum_out=racc2)
        nc.vector.tensor_add(racc, racc, racc2)
        nc.vector.tensor_add(racc, racc, racc3)
        rsum_g = group_reduce(racc, 1)
        negsum = small.tile([P, 1], f32, tag="negsum")
        nc.vector.tensor_mul(negsum, kk, tstar)
        nc.vector.tensor_add(negsum, negsum, rsum_g)

        # final losses + output
        denom = small.tile([P, 1], f32, tag="denom")
        invd = small.tile([P, 1], f32, tag="invd")
        nc.vector.tensor_scalar(out=denom, in0=np128, scalar1=1.0, scalar2=None,
                                op0=Alu.max)
        nc.vector.reciprocal(out=invd, in_=denom)
        out128 = small.tile([P, 2], f32, tag="out128")
        nc.vector.tensor_mul(out128[:, 0:1], pos128, invd)
        nc.vector.tensor_mul(out128[:, 1:2], negsum, invd)
        pstride = out128.ap[0][0]
        src = bass.AP(tensor=out128.tensor, offset=out128.offset,
                      ap=[[J * pstride, S], [1, 2]])
        nc.sync.dma_start(out=out_d.ap(), in_=src)

    nc.compile()
    return nc


def _get_nc():
    if "nc" not in _CACHE:
        _CACHE["nc"] = _build()
    return _CACHE["nc"]


def kernel(pred, target, mask_ignore, neg_rand):
    from concourse.bass_utils import run_bass_kernel_spmd

    nc = _get_nc()
    pred = np.ascontiguousarray(np.asarray(pred, dtype=np.float32).reshape(B, N))
    target = np.ascontiguousarray(np.asarray(target, dtype=np.float32).reshape(B, N))
    mask_ignore = np.ascontiguousarray(
        np.asarray(mask_ignore, dtype=np.float32).reshape(B, N))
    neg_rand = np.ascontiguousarray(
        np.asarray(neg_rand, dtype=np.float32).reshape(B, N))

    n_cores = B // S
    in_maps = []
    for c in range(n_cores):
        sl = slice(c * S, (c + 1) * S)
        in_maps.append({
            "x": pred[sl], "t": target[sl], "g": mask_ignore[sl], "u": neg_rand[sl],
        })
    res = run_bass_kernel_spmd(nc, in_maps, core_ids=list(range(n_cores)))
    outs = np.stack([r["out"] for r in res.results])  # [cores, S, 2]
    pos = np.float32(outs[:, :, 0].sum(dtype=np.float64) / B)
    neg = np.float32(outs[:, :, 1].sum(dtype=np.float64) / B)
    return pos, neg
